# revision 19
# baseline (speedup 1.0000x reference)
"""Grouped per-sample MLP (conv1d groups=B) + GroupSwish + softmax, on 8 NeuronCores.

Data-parallel over the group axis B=256: 32 groups per core, processed in
8 quads of 4 groups. Per group g: h = W1[g] @ x[g] + b1[g]; GroupSwish;
o = W2[g] @ h + b2[g]; softmax over the flattened [C*L] logits.

Key design points (vs the fp32 baseline at ~312us):
  - x is shipped as fp8e4m3 (12.8 MB/core), W1/W2 as bf16. End-to-end
    rel_fro error ~6.5e-3 (numpy-simulated), well under the 2e-2 gate.
  - Contraction X=784 is split 7x112 (not 6x128+16) so every matmul is
    K<=128, M=32: uniform (128,32) PE tile mode -> no mode-switch drains.
  - 4 groups share the 128-wide PE array via column tiling: matmuls for
    the 4 groups of a quad write PSUM partition strips 32j..32j+31 and
    run concurrently (tile_position auto-derived from out.base_partition).
    Emission is chunk-outer / group-inner so the 4 strips stay busy.
  - Activations/DVE ops run on whole [128, 512] quads (DVE/ACT cost is
    per-free-dim-element, not per-partition, so 4 groups cost 1 group).
  - W2 per group is embedded as a [128, 32] column block (rows 32j..+31
    hold W2[g].T/1.1, rest zero) -> o quad in one (128,32)-mode pass;
    pad logit rows compute as exactly 0.
  - Softmax cross-partition sum + broadcast with two (128,32)-mode
    matmuls against constant selector matrices; garbage is never
    multiplied by 0 anywhere (no NaN paths).
  - x DMA per quad is split across both HWDGE rings (sync + scalar),
    one contiguous 7168B run per partition.
"""

import os
import numpy as np
import ml_dtypes
from contextlib import ExitStack

import concourse.mybir as mybir
import concourse.tile as tile
from concourse import bacc
from concourse.bass_utils import run_bass_kernel_spmd

B, X, Z, C, L = 256, 784, 32, 10, 512
NCORE = 8
GPC = B // NCORE  # 32 groups per core
NQ = GPC // 4  # 8 quads of 4 groups
NCH = 7  # contraction chunks
KP = X // NCH  # 112 rows per chunk
F32 = mybir.dt.float32
BF16 = mybir.dt.bfloat16
FP8 = mybir.dt.float8e4

NP_BF16 = ml_dtypes.bfloat16
NP_FP8 = ml_dtypes.float8_e4m3fn

DEFAULT_CFG = dict(
    x_bufs=8,
    s_bufs=3,
    h_bufs=3,
    o_bufs=2,
    warmup=20,
)

_CACHE: dict = {}


def _build(cfg=DEFAULT_CFG):
    nc = bacc.Bacc("TRN2", target_bir_lowering=False, debug=False)

    xm = nc.dram_tensor("xm", [NQ, KP, 4 * NCH * L], FP8, kind="ExternalInput").ap()
    w1m = nc.dram_tensor("w1m", [KP, NQ * 4 * NCH * Z], BF16, kind="ExternalInput").ap()
    w2c = nc.dram_tensor("w2c", [128, NQ * 4 * Z], BF16, kind="ExternalInput").ap()
    onest = nc.dram_tensor("onest", [128, 4 * Z], BF16, kind="ExternalInput").ap()
    sphq = nc.dram_tensor("sphq", [128, NQ], F32, kind="ExternalInput").ap()
    spbq = nc.dram_tensor("spbq", [128, NQ], F32, kind="ExternalInput").ap()
    b1q = nc.dram_tensor("b1q", [128, NQ], F32, kind="ExternalInput").ap()
    b2q = nc.dram_tensor("b2q", [128, NQ], F32, kind="ExternalInput").ap()
    out = nc.dram_tensor("out", [GPC, C, L], BF16, kind="ExternalOutput").ap()

    with tile.TileContext(nc) as tc, ExitStack() as ctx:
        consts = ctx.enter_context(tc.tile_pool(name="consts", bufs=1))
        xpool = ctx.enter_context(tc.tile_pool(name="x", bufs=cfg["x_bufs"]))
        spool = ctx.enter_context(tc.tile_pool(name="act", bufs=cfg["s_bufs"]))
        hps = ctx.enter_context(tc.tile_pool(name="hps", bufs=cfg["h_bufs"], space="PSUM"))
        ops = ctx.enter_context(tc.tile_pool(name="ops", bufs=cfg["o_bufs"], space="PSUM"))
        tps = ctx.enter_context(tc.tile_pool(name="tps", bufs=2, space="PSUM"))

        w2t = consts.tile([128, NQ * 4 * Z], BF16, name="w2t")
        nc.gpsimd.dma_start(w2t[:], w2c)
        ot = consts.tile([128, 4 * Z], BF16, name="ot")
        nc.gpsimd.dma_start(ot[:], onest)
        spht = consts.tile([128, NQ], F32, name="spht")
        nc.gpsimd.dma_start(spht[:], sphq)
        spbt = consts.tile([128, NQ], F32, name="spbt")
        nc.gpsimd.dma_start(spbt[:], spbq)
        b1t = consts.tile([128, NQ], F32, name="b1t")
        nc.gpsimd.dma_start(b1t[:], b1q)
        b2t = consts.tile([128, NQ], F32, name="b2t")
        nc.gpsimd.dma_start(b2t[:], b2q)

        # all of W1 up front (1.6 MB): one DMA, first in the sync ring
        wall = consts.tile([128, NQ * 4 * NCH * Z], BF16, name="wall")
        nc.sync.dma_start(wall[:KP, :], w1m)

        # PE warm-up: ~20 dummy matmuls on uninitialized SBUF into a
        # scratch PSUM bank. No deps -> runs immediately; pushes the HAM
        # past its 4096-cycle activity window so real matmuls run at
        # 2.4 GHz instead of 1.2.
        scr = consts.tile([128, L], FP8, name="scr")
        nc.vector.memset(scr[:], 0.5)
        scw = consts.tile([128, Z], BF16, name="scw")
        nc.vector.memset(scw[:], 0.5)
        wps = ctx.enter_context(tc.tile_pool(name="wps", bufs=1, space="PSUM"))
        warm = wps.tile([Z, L], F32, name="warm")
        for i in range(cfg.get("warmup", 20)):
            nc.tensor.matmul(warm[:], scw[:], scr[:], start=True, stop=True)

        for q in range(NQ):
            # --- loads ---
            xt = xpool.tile([128, 4 * NCH * L], FP8, tag="xt", name=f"xt{q}")
            half = 2 * NCH * L
            nc.sync.dma_start(xt[:KP, :half], xm[q, :, :half])
            nc.scalar.dma_start(xt[:KP, half:], xm[q, :, half:])
            wq0 = q * 4 * NCH * Z

            # --- h quad: 4 groups x [32, 512], col-tiled, chunk-outer ---
            h4 = hps.tile([128, L], F32, tag="h", name=f"h{q}")
            for c in range(NCH):
                for j in range(4):
                    s = (j * NCH + c)
                    nc.tensor.matmul(
                        h4[32 * j : 32 * j + 32, :],
                        wall[:KP, wq0 + s * Z : wq0 + (s + 1) * Z],
                        xt[:KP, s * L : (s + 1) * L],
                        start=(c == 0),
                        stop=(c == NCH - 1),
                        tile_position=(0, 32 * j),
                    )

            # --- GroupSwish: ((h+b1)*0.5) * (1 + tanh(sp/2*(h+b1))) ---
            t = spool.tile([128, L], BF16, tag="t", name=f"t{q}")
            nc.scalar.activation(
                t[:],
                h4[:],
                mybir.ActivationFunctionType.Tanh,
                bias=spbt[:, q : q + 1],
                scale=spht[:, q : q + 1],
            )
            u = spool.tile([128, L], BF16, tag="u", name=f"u{q}")
            nc.vector.tensor_scalar(
                u[:],
                h4[:],
                b1t[:, q : q + 1],
                0.5,
                op0=mybir.AluOpType.add,
                op1=mybir.AluOpType.mult,
            )
            sw = spool.tile([128, L], BF16, tag="sw", name=f"sw{q}")
            nc.vector.scalar_tensor_tensor(
                sw[:],
                t[:],
                1.0,
                u[:],
                op0=mybir.AluOpType.add,
                op1=mybir.AluOpType.mult,
            )

            # --- o quad: 4 groups x [32(10 used), 512] via zero-padded W2 blocks ---
            o4 = ops.tile([128, L], F32, tag="o", name=f"o{q}")
            for j in range(4):
                g = 4 * q + j
                nc.tensor.matmul(
                    o4[32 * j : 32 * j + 32, :],
                    w2t[:, g * Z : (g + 1) * Z],
                    sw[:],
                    start=True,
                    stop=True,
                    tile_position=(0, 32 * j),
                )

            # --- softmax ---
            expo = spool.tile([128, L], BF16, tag="expo", name=f"e{q}")
            esum = spool.tile([128, 1], F32, tag="esum", name=f"es{q}")
            nc.scalar.activation(
                expo[:],
                o4[:],
                mybir.ActivationFunctionType.Exp,
                bias=b2t[:, q : q + 1],
                scale=1.0,
                accum_out=esum[:],
            )
            esb = spool.tile([128, 1], BF16, tag="esb", name=f"eb{q}")
            nc.vector.tensor_copy(esb[:], esum[:])
            # totb[32j+c] = sum of esum over the 10 logit rows of group j
            totb = tps.tile([128, 1], F32, tag="tb", name=f"tot{q}")
            for j in range(4):
                nc.tensor.matmul(
                    totb[32 * j : 32 * j + 32, :],
                    ot[:, 32 * j : 32 * j + 32],
                    esb[:],
                    start=True,
                    stop=True,
                    tile_position=(0, 32 * j),
                )
            invb = spool.tile([128, 1], F32, tag="invb", name=f"iv{q}")
            nc.vector.reciprocal(invb[:], totb[:])
            res = spool.tile([128, L], BF16, tag="res", name=f"r{q}")
            nc.vector.tensor_scalar_mul(res[:], expo[:], invb[:])
            for j in range(4):
                nc.gpsimd.dma_start(out[4 * q + j], res[32 * j : 32 * j + 10, :])

    nc.compile()
    return nc


def _marshal(x, W1, b1, beta, W2, b2):
    """Full inputs -> list of per-core input dicts (all layouts hardcoded)."""
    # x: [1, B*X, L] -> [B, 7, 112, L] -> per-quad partition-major fp8
    xg = np.asarray(x, dtype=np.float32).reshape(B, NCH, KP, L)
    x8 = xg.astype(NP_FP8)
    # [B/4 quads, 4, NCH, KP, L] -> [quads, KP, 4, NCH, L]
    x8 = x8.reshape(B // 4, 4, NCH, KP, L).transpose(0, 3, 1, 2, 4)
    x8 = np.ascontiguousarray(x8).reshape(B // 4, KP, 4 * NCH * L)

    # W1: [B, Z, X] -> lhsT chunks, partition-major [KP, (quad, j, c, z)] bf16
    w1T = np.asarray(W1, dtype=np.float32).transpose(0, 2, 1)  # [B, X, Z]
    w1c = w1T.reshape(B // 4, 4, NCH, KP, Z).transpose(3, 0, 1, 2, 4)
    w1c = np.ascontiguousarray(w1c).astype(NP_BF16).reshape(KP, (B // 4) * 4 * NCH * Z)

    # W2 blockdiag: w2c[32j+z, g*Z+c-block] = W2[g, c, z]/1.1 (per core below)
    w2s = (np.asarray(W2, dtype=np.float32) * np.float32(1.0 / 1.1)).transpose(0, 2, 1)  # [B, Z, C]

    onest = np.zeros((128, 4 * Z), dtype=NP_BF16)
    for j in range(4):
        onest[32 * j : 32 * j + C, 32 * j : 32 * j + 32] = NP_BF16(1.0)

    b1f = np.asarray(b1, dtype=np.float32)
    b2f = np.asarray(b2, dtype=np.float32)
    spf = np.log1p(np.exp(np.asarray(beta, dtype=np.float64))).astype(np.float32)

    in_maps = []
    for core in range(NCORE):
        g0 = core * GPC
        sq = slice(core * NQ, (core + 1) * NQ)

        w2core = np.zeros((128, NQ * 4 * Z), dtype=np.float32)
        sph = np.zeros((128, NQ), dtype=np.float32)
        spb = np.zeros((128, NQ), dtype=np.float32)
        b1m = np.zeros((128, NQ), dtype=np.float32)
        b2m = np.zeros((128, NQ), dtype=np.float32)
        for q in range(NQ):
            for j in range(4):
                g = g0 + 4 * q + j
                w2core[32 * j : 32 * j + Z, (4 * q + j) * Z : (4 * q + j) * Z + C] = w2s[g]
                sph[32 * j : 32 * j + Z, q] = 0.5 * spf[g]
                spb[32 * j : 32 * j + Z, q] = 0.5 * spf[g] * b1f[g]
                b1m[32 * j : 32 * j + Z, q] = b1f[g]
                b2m[32 * j : 32 * j + C, q] = b2f[g]

        wstep = NQ * 4 * NCH * Z
        in_maps.append(
            {
                "xm": x8[sq],
                "w1m": np.ascontiguousarray(
                    w1c[:, core * wstep : (core + 1) * wstep]
                ),
                "w2c": w2core.astype(NP_BF16),
                "onest": onest,
                "sphq": sph,
                "spbq": spb,
                "b1q": b1m,
                "b2q": b2m,
            }
        )
    return in_maps


def _run(in_maps, cfg=DEFAULT_CFG, trace=False, tmpdir=None):
    key = str(sorted(cfg.items()))
    if key not in _CACHE:
        _CACHE[key] = _build(cfg)
    return run_bass_kernel_spmd(
        _CACHE[key],
        in_maps,
        core_ids=list(range(NCORE)),
        trace=trace,
        tmpdir=tmpdir,
    )


_LAST = {}


def kernel(x, W1, b1, beta, W2, b2):
    in_maps = _marshal(x, W1, b1, beta, W2, b2)
    trace = bool(os.environ.get("KERNEL_TRACE"))
    r = _run(in_maps, trace=trace, tmpdir=os.environ.get("KERNEL_TRACE_DIR"))
    _LAST["results"] = r
    outs = [
        r.results[c]["out"].astype(np.float32).reshape(GPC, C * L)
        for c in range(NCORE)
    ]
    return np.concatenate(outs, axis=0)


# revision 20
# speedup vs baseline: 1.0427x; 1.0427x over previous
"""Grouped per-sample MLP (conv1d groups=B) + GroupSwish + softmax, on 8 NeuronCores.

Data-parallel over the group axis B=256: 32 groups per core, processed in
8 quads of 4 groups. Per group g: h = W1[g] @ x[g] + b1[g]; GroupSwish;
o = W2[g] @ h + b2[g]; softmax over the flattened [C*L] logits.

Key design points (vs the fp32 baseline at ~312us):
  - x is shipped as fp8e4m3 (12.8 MB/core), W1/W2 as bf16. End-to-end
    rel_fro error ~6.5e-3 (numpy-simulated), well under the 2e-2 gate.
  - Contraction X=784 is split 7x112 (not 6x128+16) so every matmul is
    K<=128, M=32: uniform (128,32) PE tile mode -> no mode-switch drains.
  - 4 groups share the 128-wide PE array via column tiling: matmuls for
    the 4 groups of a quad write PSUM partition strips 32j..32j+31 and
    run concurrently (tile_position auto-derived from out.base_partition).
    Emission is chunk-outer / group-inner so the 4 strips stay busy.
  - Activations/DVE ops run on whole [128, 512] quads (DVE/ACT cost is
    per-free-dim-element, not per-partition, so 4 groups cost 1 group).
  - W2 per group is embedded as a [128, 32] column block (rows 32j..+31
    hold W2[g].T/1.1, rest zero) -> o quad in one (128,32)-mode pass;
    pad logit rows compute as exactly 0.
  - Softmax cross-partition sum + broadcast with two (128,32)-mode
    matmuls against constant selector matrices; garbage is never
    multiplied by 0 anywhere (no NaN paths).
  - x DMA per quad is split across both HWDGE rings (sync + scalar),
    one contiguous 7168B run per partition.
"""

import os
import numpy as np
import ml_dtypes
from contextlib import ExitStack

import concourse.mybir as mybir
import concourse.tile as tile
from concourse import bacc
from concourse.bass_utils import run_bass_kernel_spmd

B, X, Z, C, L = 256, 784, 32, 10, 512
NCORE = 8
GPC = B // NCORE  # 32 groups per core
NQ = GPC // 4  # 8 quads of 4 groups
NCH = 7  # contraction chunks
KP = X // NCH  # 112 rows per chunk
F32 = mybir.dt.float32
BF16 = mybir.dt.bfloat16
FP8 = mybir.dt.float8e4

NP_BF16 = ml_dtypes.bfloat16
NP_FP8 = ml_dtypes.float8_e4m3fn

DEFAULT_CFG = dict(
    x_bufs=8,
    s_bufs=3,
    h_bufs=3,
    o_bufs=2,
    warmup=20,
)

_CACHE: dict = {}


def _build(cfg=DEFAULT_CFG):
    nc = bacc.Bacc("TRN2", target_bir_lowering=False, debug=False)

    xm = nc.dram_tensor("xm", [NQ, KP, 4 * NCH * L], FP8, kind="ExternalInput").ap()
    w1m = nc.dram_tensor("w1m", [KP, NQ * 4 * NCH * Z], BF16, kind="ExternalInput").ap()
    w2c = nc.dram_tensor("w2c", [128, NQ * 4 * Z], BF16, kind="ExternalInput").ap()
    onest = nc.dram_tensor("onest", [128, 4 * Z], BF16, kind="ExternalInput").ap()
    sphq = nc.dram_tensor("sphq", [128, NQ], F32, kind="ExternalInput").ap()
    spbq = nc.dram_tensor("spbq", [128, NQ], F32, kind="ExternalInput").ap()
    b1q = nc.dram_tensor("b1q", [128, NQ], F32, kind="ExternalInput").ap()
    b2q = nc.dram_tensor("b2q", [128, NQ], F32, kind="ExternalInput").ap()
    out = nc.dram_tensor("out", [GPC, C, L], BF16, kind="ExternalOutput").ap()

    with tile.TileContext(nc) as tc, ExitStack() as ctx:
        consts = ctx.enter_context(tc.tile_pool(name="consts", bufs=1))
        xpool = ctx.enter_context(tc.tile_pool(name="x", bufs=cfg["x_bufs"]))
        spool = ctx.enter_context(tc.tile_pool(name="act", bufs=cfg["s_bufs"]))
        hps = ctx.enter_context(tc.tile_pool(name="hps", bufs=cfg["h_bufs"], space="PSUM"))
        ops = ctx.enter_context(tc.tile_pool(name="ops", bufs=cfg["o_bufs"], space="PSUM"))
        tps = ctx.enter_context(tc.tile_pool(name="tps", bufs=2, space="PSUM"))

        w2t = consts.tile([128, NQ * 4 * Z], BF16, name="w2t")
        nc.gpsimd.dma_start(w2t[:], w2c)
        ot = consts.tile([128, 4 * Z], BF16, name="ot")
        nc.gpsimd.dma_start(ot[:], onest)
        spht = consts.tile([128, NQ], F32, name="spht")
        nc.gpsimd.dma_start(spht[:], sphq)
        spbt = consts.tile([128, NQ], F32, name="spbt")
        nc.gpsimd.dma_start(spbt[:], spbq)
        b1t = consts.tile([128, NQ], F32, name="b1t")
        nc.gpsimd.dma_start(b1t[:], b1q)
        b2t = consts.tile([128, NQ], F32, name="b2t")
        nc.gpsimd.dma_start(b2t[:], b2q)

        # all of W1 up front (1.6 MB): one DMA, first in the sync ring
        wall = consts.tile([128, NQ * 4 * NCH * Z], BF16, name="wall")
        nc.sync.dma_start(wall[:KP, :], w1m)

        # PE warm-up: ~20 dummy matmuls on uninitialized SBUF into a
        # scratch PSUM bank. No deps -> runs immediately; pushes the HAM
        # past its 4096-cycle activity window so real matmuls run at
        # 2.4 GHz instead of 1.2.
        scr = consts.tile([128, L], FP8, name="scr")
        nc.vector.memset(scr[:], 0.5)
        scw = consts.tile([128, Z], BF16, name="scw")
        nc.vector.memset(scw[:], 0.5)
        wps = ctx.enter_context(tc.tile_pool(name="wps", bufs=1, space="PSUM"))
        warm = wps.tile([Z, L], F32, name="warm")
        for i in range(cfg.get("warmup", 20)):
            nc.tensor.matmul(warm[:], scw[:], scr[:], start=True, stop=True)

        sws = {}  # q -> swish tile
        expos = {}  # q -> (expo, esb)

        def stage1(q):
            """loads + W1 matmuls + GroupSwish for quad q."""
            xt = xpool.tile([128, 4 * NCH * L], FP8, tag="xt", name=f"xt{q}")
            c1 = 12 * L  # 3/7 of the quad to each HWDGE ring
            c2 = 24 * L
            nc.sync.dma_start(xt[:KP, :c1], xm[q, :, :c1])
            nc.scalar.dma_start(xt[:KP, c1:c2], xm[q, :, c1:c2])
            nc.gpsimd.dma_start(xt[:KP, c2:], xm[q, :, c2:])
            wq0 = q * 4 * NCH * Z

            h4 = hps.tile([128, L], F32, tag="h", name=f"h{q}")
            for c in range(NCH):
                for j in range(4):
                    s = (j * NCH + c)
                    nc.tensor.matmul(
                        h4[32 * j : 32 * j + 32, :],
                        wall[:KP, wq0 + s * Z : wq0 + (s + 1) * Z],
                        xt[:KP, s * L : (s + 1) * L],
                        start=(c == 0),
                        stop=(c == NCH - 1),
                        tile_position=(0, 32 * j),
                    )

            t = spool.tile([128, L], BF16, tag="t", name=f"t{q}")
            nc.scalar.activation(
                t[:],
                h4[:],
                mybir.ActivationFunctionType.Tanh,
                bias=spbt[:, q : q + 1],
                scale=spht[:, q : q + 1],
            )
            u = spool.tile([128, L], BF16, tag="u", name=f"u{q}")
            nc.vector.tensor_scalar(
                u[:],
                h4[:],
                b1t[:, q : q + 1],
                0.5,
                op0=mybir.AluOpType.add,
                op1=mybir.AluOpType.mult,
            )
            sw = spool.tile([128, L], BF16, tag="sw", name=f"sw{q}")
            nc.vector.scalar_tensor_tensor(
                sw[:],
                t[:],
                1.0,
                u[:],
                op0=mybir.AluOpType.add,
                op1=mybir.AluOpType.mult,
            )
            sws[q] = sw

        def stage2(q):
            """W2 matmuls + exp for quad q (emitted one quad later)."""
            o4 = ops.tile([128, L], F32, tag="o", name=f"o{q}")
            sw = sws.pop(q)
            for j in range(4):
                g = 4 * q + j
                nc.tensor.matmul(
                    o4[32 * j : 32 * j + 32, :],
                    w2t[:, g * Z : (g + 1) * Z],
                    sw[:],
                    start=True,
                    stop=True,
                    tile_position=(0, 32 * j),
                )
            expo = spool.tile([128, L], BF16, tag="expo", name=f"e{q}")
            esum = spool.tile([128, 1], F32, tag="esum", name=f"es{q}")
            nc.scalar.activation(
                expo[:],
                o4[:],
                mybir.ActivationFunctionType.Exp,
                bias=b2t[:, q : q + 1],
                scale=1.0,
                accum_out=esum[:],
            )
            esb = spool.tile([128, 1], BF16, tag="esb", name=f"eb{q}")
            nc.vector.tensor_copy(esb[:], esum[:])
            expos[q] = (expo, esb)

        def stage3(q):
            """softmax normalization + store for quad q (two quads later)."""
            expo, esb = expos.pop(q)
            totb = tps.tile([128, 1], F32, tag="tb", name=f"tot{q}")
            for j in range(4):
                nc.tensor.matmul(
                    totb[32 * j : 32 * j + 32, :],
                    ot[:, 32 * j : 32 * j + 32],
                    esb[:],
                    start=True,
                    stop=True,
                    tile_position=(0, 32 * j),
                )
            invb = spool.tile([128, 1], F32, tag="invb", name=f"iv{q}")
            nc.vector.reciprocal(invb[:], totb[:])
            res = spool.tile([128, L], BF16, tag="res", name=f"r{q}")
            nc.vector.tensor_scalar_mul(res[:], expo[:], invb[:])
            for j in range(4):
                nc.gpsimd.dma_start(out[4 * q + j], res[32 * j : 32 * j + 10, :])

        for q in range(NQ):
            stage1(q)
            if q >= 1:
                stage2(q - 1)
            if q >= 2:
                stage3(q - 2)
        stage2(NQ - 1)
        stage3(NQ - 2)
        stage3(NQ - 1)

    nc.compile()
    return nc


def _marshal(x, W1, b1, beta, W2, b2):
    """Full inputs -> list of per-core input dicts (all layouts hardcoded)."""
    # x: [1, B*X, L] -> [B, 7, 112, L] -> per-quad partition-major fp8
    xg = np.asarray(x, dtype=np.float32).reshape(B, NCH, KP, L)
    x8 = xg.astype(NP_FP8)
    # [B/4 quads, 4, NCH, KP, L] -> [quads, KP, 4, NCH, L]
    x8 = x8.reshape(B // 4, 4, NCH, KP, L).transpose(0, 3, 1, 2, 4)
    x8 = np.ascontiguousarray(x8).reshape(B // 4, KP, 4 * NCH * L)

    # W1: [B, Z, X] -> lhsT chunks, partition-major [KP, (quad, j, c, z)] bf16
    w1T = np.asarray(W1, dtype=np.float32).transpose(0, 2, 1)  # [B, X, Z]
    w1c = w1T.reshape(B // 4, 4, NCH, KP, Z).transpose(3, 0, 1, 2, 4)
    w1c = np.ascontiguousarray(w1c).astype(NP_BF16).reshape(KP, (B // 4) * 4 * NCH * Z)

    # W2 blockdiag: w2c[32j+z, g*Z+c-block] = W2[g, c, z]/1.1 (per core below)
    w2s = (np.asarray(W2, dtype=np.float32) * np.float32(1.0 / 1.1)).transpose(0, 2, 1)  # [B, Z, C]

    onest = np.zeros((128, 4 * Z), dtype=NP_BF16)
    for j in range(4):
        onest[32 * j : 32 * j + C, 32 * j : 32 * j + 32] = NP_BF16(1.0)

    b1f = np.asarray(b1, dtype=np.float32)
    b2f = np.asarray(b2, dtype=np.float32)
    spf = np.log1p(np.exp(np.asarray(beta, dtype=np.float64))).astype(np.float32)

    in_maps = []
    for core in range(NCORE):
        g0 = core * GPC
        sq = slice(core * NQ, (core + 1) * NQ)

        w2core = np.zeros((128, NQ * 4 * Z), dtype=np.float32)
        sph = np.zeros((128, NQ), dtype=np.float32)
        spb = np.zeros((128, NQ), dtype=np.float32)
        b1m = np.zeros((128, NQ), dtype=np.float32)
        b2m = np.zeros((128, NQ), dtype=np.float32)
        for q in range(NQ):
            for j in range(4):
                g = g0 + 4 * q + j
                w2core[32 * j : 32 * j + Z, (4 * q + j) * Z : (4 * q + j) * Z + C] = w2s[g]
                sph[32 * j : 32 * j + Z, q] = 0.5 * spf[g]
                spb[32 * j : 32 * j + Z, q] = 0.5 * spf[g] * b1f[g]
                b1m[32 * j : 32 * j + Z, q] = b1f[g]
                b2m[32 * j : 32 * j + C, q] = b2f[g]

        wstep = NQ * 4 * NCH * Z
        in_maps.append(
            {
                "xm": x8[sq],
                "w1m": np.ascontiguousarray(
                    w1c[:, core * wstep : (core + 1) * wstep]
                ),
                "w2c": w2core.astype(NP_BF16),
                "onest": onest,
                "sphq": sph,
                "spbq": spb,
                "b1q": b1m,
                "b2q": b2m,
            }
        )
    return in_maps


def _run(in_maps, cfg=DEFAULT_CFG, trace=False, tmpdir=None):
    key = str(sorted(cfg.items()))
    if key not in _CACHE:
        _CACHE[key] = _build(cfg)
    return run_bass_kernel_spmd(
        _CACHE[key],
        in_maps,
        core_ids=list(range(NCORE)),
        trace=trace,
        tmpdir=tmpdir,
    )


_LAST = {}


def kernel(x, W1, b1, beta, W2, b2):
    in_maps = _marshal(x, W1, b1, beta, W2, b2)
    trace = bool(os.environ.get("KERNEL_TRACE"))
    r = _run(in_maps, trace=trace, tmpdir=os.environ.get("KERNEL_TRACE_DIR"))
    _LAST["results"] = r
    outs = [
        r.results[c]["out"].astype(np.float32).reshape(GPC, C * L)
        for c in range(NCORE)
    ]
    return np.concatenate(outs, axis=0)


# revision 23
# speedup vs baseline: 1.0631x; 1.0196x over previous
"""Grouped per-sample MLP (conv1d groups=B) + GroupSwish + softmax, on 8 NeuronCores.

Data-parallel over the group axis B=256: 32 groups per core, processed in
8 quads of 4 groups. Per group g: h = W1[g] @ x[g] + b1[g]; GroupSwish;
o = W2[g] @ h + b2[g]; softmax over the flattened [C*L] logits.

Key design points (vs the fp32 baseline at ~312us):
  - x is shipped as fp8e4m3 (12.8 MB/core), W1/W2 as bf16. End-to-end
    rel_fro error ~6.5e-3 (numpy-simulated), well under the 2e-2 gate.
  - Contraction X=784 is split 7x112 (not 6x128+16) so every matmul is
    K<=128, M=32: uniform (128,32) PE tile mode -> no mode-switch drains.
  - 4 groups share the 128-wide PE array via column tiling: matmuls for
    the 4 groups of a quad write PSUM partition strips 32j..32j+31 and
    run concurrently (tile_position auto-derived from out.base_partition).
    Emission is chunk-outer / group-inner so the 4 strips stay busy.
  - Activations/DVE ops run on whole [128, 512] quads (DVE/ACT cost is
    per-free-dim-element, not per-partition, so 4 groups cost 1 group).
  - W2 per group is embedded as a [128, 32] column block (rows 32j..+31
    hold W2[g].T/1.1, rest zero) -> o quad in one (128,32)-mode pass;
    pad logit rows compute as exactly 0.
  - Softmax cross-partition sum + broadcast with two (128,32)-mode
    matmuls against constant selector matrices; garbage is never
    multiplied by 0 anywhere (no NaN paths).
  - x DMA per quad is split across both HWDGE rings (sync + scalar),
    one contiguous 7168B run per partition.
"""

import os
import numpy as np
import ml_dtypes
from contextlib import ExitStack

import concourse.mybir as mybir
import concourse.tile as tile
from concourse import bacc
from concourse.bass_utils import run_bass_kernel_spmd

B, X, Z, C, L = 256, 784, 32, 10, 512
NCORE = 8
GPC = B // NCORE  # 32 groups per core
NQ = GPC // 4  # 8 quads of 4 groups
NCH = 7  # contraction chunks
KP = X // NCH  # 112 rows per chunk
F32 = mybir.dt.float32
BF16 = mybir.dt.bfloat16
FP8 = mybir.dt.float8e4

NP_BF16 = ml_dtypes.bfloat16
NP_FP8 = ml_dtypes.float8_e4m3fn

DEFAULT_CFG = dict(
    x_bufs=8,
    s_bufs=3,
    h_bufs=3,
    o_bufs=2,
    warmup=20,
)

_CACHE: dict = {}


def _build(cfg=DEFAULT_CFG):
    nc = bacc.Bacc("TRN2", target_bir_lowering=False, debug=False)

    xm = nc.dram_tensor("xm", [NQ, NCH * KP, 4 * L], FP8, kind="ExternalInput").ap()
    w1m = nc.dram_tensor("w1m", [KP, NQ * 4 * NCH * Z], BF16, kind="ExternalInput").ap()
    w2c = nc.dram_tensor("w2c", [128, NQ * 4 * Z], BF16, kind="ExternalInput").ap()
    onest = nc.dram_tensor("onest", [128, 4 * Z], BF16, kind="ExternalInput").ap()
    sphq = nc.dram_tensor("sphq", [128, NQ], F32, kind="ExternalInput").ap()
    spbq = nc.dram_tensor("spbq", [128, NQ], F32, kind="ExternalInput").ap()
    b1q = nc.dram_tensor("b1q", [128, NQ], F32, kind="ExternalInput").ap()
    b2q = nc.dram_tensor("b2q", [128, NQ], F32, kind="ExternalInput").ap()
    out = nc.dram_tensor("out", [GPC, C, L], BF16, kind="ExternalOutput").ap()

    with tile.TileContext(nc) as tc, ExitStack() as ctx:
        consts = ctx.enter_context(tc.tile_pool(name="consts", bufs=1))
        xpool = ctx.enter_context(tc.tile_pool(name="x", bufs=cfg["x_bufs"]))
        spool = ctx.enter_context(tc.tile_pool(name="act", bufs=cfg["s_bufs"]))
        hps = ctx.enter_context(tc.tile_pool(name="hps", bufs=cfg["h_bufs"], space="PSUM"))
        ops = ctx.enter_context(tc.tile_pool(name="ops", bufs=cfg["o_bufs"], space="PSUM"))
        tps = ctx.enter_context(tc.tile_pool(name="tps", bufs=2, space="PSUM"))

        w2t = consts.tile([128, NQ * 4 * Z], BF16, name="w2t")
        nc.gpsimd.dma_start(w2t[:], w2c)
        ot = consts.tile([128, 4 * Z], BF16, name="ot")
        nc.gpsimd.dma_start(ot[:], onest)
        spht = consts.tile([128, NQ], F32, name="spht")
        nc.gpsimd.dma_start(spht[:], sphq)
        spbt = consts.tile([128, NQ], F32, name="spbt")
        nc.gpsimd.dma_start(spbt[:], spbq)
        b1t = consts.tile([128, NQ], F32, name="b1t")
        nc.gpsimd.dma_start(b1t[:], b1q)
        b2t = consts.tile([128, NQ], F32, name="b2t")
        nc.gpsimd.dma_start(b2t[:], b2q)

        # all of W1 up front (1.6 MB): one DMA, first in the sync ring
        wall = consts.tile([128, NQ * 4 * NCH * Z], BF16, name="wall")
        nc.sync.dma_start(wall[:KP, :], w1m)

        # PE warm-up: ~20 dummy matmuls on uninitialized SBUF into a
        # scratch PSUM bank. No deps -> runs immediately; pushes the HAM
        # past its 4096-cycle activity window so real matmuls run at
        # 2.4 GHz instead of 1.2.
        scr = consts.tile([128, L], FP8, name="scr")
        nc.vector.memset(scr[:], 0.5)
        scw = consts.tile([128, Z], BF16, name="scw")
        nc.vector.memset(scw[:], 0.5)
        wps = ctx.enter_context(tc.tile_pool(name="wps", bufs=1, space="PSUM"))
        warm = wps.tile([Z, L], F32, name="warm")
        for i in range(cfg.get("warmup", 20)):
            nc.tensor.matmul(warm[:], scw[:], scr[:], start=True, stop=True)

        sws = {}  # q -> swish tile
        expos = {}  # q -> (expo, esb)

        def stage1(q):
            """loads + W1 matmuls + GroupSwish for quad q."""
            # x tile layout: [p, (c, j, l)] -> rhs slice for (c, j) is
            # xt[:KP, (c*4+j)*L : +L]. DRAM is chunk-major [(c p), (j l)]
            # so every partition receives 7 runs of 2048 B (one per chunk).
            xt = xpool.tile([128, 4 * NCH * L], FP8, tag="xt", name=f"xt{q}")
            CS = 4  # chunks 0..3 on the sync ring, 4..6 on scalar
            nc.sync.dma_start(
                xt[:KP, : CS * 4 * L].rearrange("p (c v) -> p c v", c=CS),
                xm[q, : CS * KP].rearrange("(c p) v -> p c v", p=KP),
            )
            nc.scalar.dma_start(
                xt[:KP, CS * 4 * L :].rearrange("p (c v) -> p c v", c=NCH - CS),
                xm[q, CS * KP :].rearrange("(c p) v -> p c v", p=KP),
            )
            wq0 = q * 4 * NCH * Z

            h4 = hps.tile([128, L], F32, tag="h", name=f"h{q}")
            for c in range(NCH):
                for j in range(4):
                    s = (j * NCH + c)
                    nc.tensor.matmul(
                        h4[32 * j : 32 * j + 32, :],
                        wall[:KP, wq0 + s * Z : wq0 + (s + 1) * Z],
                        xt[:KP, (c * 4 + j) * L : (c * 4 + j + 1) * L],
                        start=(c == 0),
                        stop=(c == NCH - 1),
                        tile_position=(0, 32 * j),
                    )

            t = spool.tile([128, L], BF16, tag="t", name=f"t{q}")
            nc.scalar.activation(
                t[:],
                h4[:],
                mybir.ActivationFunctionType.Tanh,
                bias=spbt[:, q : q + 1],
                scale=spht[:, q : q + 1],
            )
            u = spool.tile([128, L], BF16, tag="u", name=f"u{q}")
            nc.vector.tensor_scalar(
                u[:],
                h4[:],
                b1t[:, q : q + 1],
                0.5,
                op0=mybir.AluOpType.add,
                op1=mybir.AluOpType.mult,
            )
            sw = spool.tile([128, L], BF16, tag="sw", name=f"sw{q}")
            nc.vector.scalar_tensor_tensor(
                sw[:],
                t[:],
                1.0,
                u[:],
                op0=mybir.AluOpType.add,
                op1=mybir.AluOpType.mult,
            )
            sws[q] = sw

        def stage2(q):
            """W2 matmuls + exp for quad q (emitted one quad later)."""
            o4 = ops.tile([128, L], F32, tag="o", name=f"o{q}")
            sw = sws.pop(q)
            for j in range(4):
                g = 4 * q + j
                nc.tensor.matmul(
                    o4[32 * j : 32 * j + 32, :],
                    w2t[:, g * Z : (g + 1) * Z],
                    sw[:],
                    start=True,
                    stop=True,
                    tile_position=(0, 32 * j),
                )
            expo = spool.tile([128, L], BF16, tag="expo", name=f"e{q}")
            esum = spool.tile([128, 1], F32, tag="esum", name=f"es{q}")
            nc.scalar.activation(
                expo[:],
                o4[:],
                mybir.ActivationFunctionType.Exp,
                bias=b2t[:, q : q + 1],
                scale=1.0,
                accum_out=esum[:],
            )
            esb = spool.tile([128, 1], BF16, tag="esb", name=f"eb{q}")
            nc.vector.tensor_copy(esb[:], esum[:])
            expos[q] = (expo, esb)

        def stage3(q):
            """softmax normalization + store for quad q (two quads later)."""
            expo, esb = expos.pop(q)
            totb = tps.tile([128, 1], F32, tag="tb", name=f"tot{q}")
            for j in range(4):
                nc.tensor.matmul(
                    totb[32 * j : 32 * j + 32, :],
                    ot[:, 32 * j : 32 * j + 32],
                    esb[:],
                    start=True,
                    stop=True,
                    tile_position=(0, 32 * j),
                )
            invb = spool.tile([128, 1], F32, tag="invb", name=f"iv{q}")
            nc.vector.reciprocal(invb[:], totb[:])
            res = spool.tile([128, L], BF16, tag="res", name=f"r{q}")
            nc.vector.tensor_scalar_mul(res[:], expo[:], invb[:])
            for j in range(4):
                nc.gpsimd.dma_start(out[4 * q + j], res[32 * j : 32 * j + 10, :])

        for q in range(NQ):
            stage1(q)
            if q >= 1:
                stage2(q - 1)
            if q >= 2:
                stage3(q - 2)
        stage2(NQ - 1)
        stage3(NQ - 2)
        stage3(NQ - 1)

    nc.compile()
    return nc


def _marshal(x, W1, b1, beta, W2, b2):
    """Full inputs -> list of per-core input dicts (all layouts hardcoded)."""
    # x: [1, B*X, L] -> [B, 7, 112, L] -> per-quad chunk-major fp8:
    # xm[q, c*KP+p, j*L+l] = x[4q+j, 112c+p, l]
    xg = np.asarray(x, dtype=np.float32).reshape(B, NCH, KP, L)
    x8 = xg.astype(NP_FP8)
    x8 = x8.reshape(B // 4, 4, NCH, KP, L).transpose(0, 2, 3, 1, 4)
    x8 = np.ascontiguousarray(x8).reshape(B // 4, NCH * KP, 4 * L)

    # W1: [B, Z, X] -> lhsT chunks, partition-major [KP, (quad, j, c, z)] bf16
    w1T = np.asarray(W1, dtype=np.float32).transpose(0, 2, 1)  # [B, X, Z]
    w1c = w1T.reshape(B // 4, 4, NCH, KP, Z).transpose(3, 0, 1, 2, 4)
    w1c = np.ascontiguousarray(w1c).astype(NP_BF16).reshape(KP, (B // 4) * 4 * NCH * Z)

    # W2 blockdiag: w2c[32j+z, g*Z+c-block] = W2[g, c, z]/1.1 (per core below)
    w2s = (np.asarray(W2, dtype=np.float32) * np.float32(1.0 / 1.1)).transpose(0, 2, 1)  # [B, Z, C]

    onest = np.zeros((128, 4 * Z), dtype=NP_BF16)
    for j in range(4):
        onest[32 * j : 32 * j + C, 32 * j : 32 * j + 32] = NP_BF16(1.0)

    b1f = np.asarray(b1, dtype=np.float32)
    b2f = np.asarray(b2, dtype=np.float32)
    spf = np.log1p(np.exp(np.asarray(beta, dtype=np.float64))).astype(np.float32)

    in_maps = []
    for core in range(NCORE):
        g0 = core * GPC
        sq = slice(core * NQ, (core + 1) * NQ)

        w2core = np.zeros((128, NQ * 4 * Z), dtype=np.float32)
        sph = np.zeros((128, NQ), dtype=np.float32)
        spb = np.zeros((128, NQ), dtype=np.float32)
        b1m = np.zeros((128, NQ), dtype=np.float32)
        b2m = np.zeros((128, NQ), dtype=np.float32)
        for q in range(NQ):
            for j in range(4):
                g = g0 + 4 * q + j
                w2core[32 * j : 32 * j + Z, (4 * q + j) * Z : (4 * q + j) * Z + C] = w2s[g]
                sph[32 * j : 32 * j + Z, q] = 0.5 * spf[g]
                spb[32 * j : 32 * j + Z, q] = 0.5 * spf[g] * b1f[g]
                b1m[32 * j : 32 * j + Z, q] = b1f[g]
                b2m[32 * j : 32 * j + C, q] = b2f[g]

        wstep = NQ * 4 * NCH * Z
        in_maps.append(
            {
                "xm": x8[sq],
                "w1m": np.ascontiguousarray(
                    w1c[:, core * wstep : (core + 1) * wstep]
                ),
                "w2c": w2core.astype(NP_BF16),
                "onest": onest,
                "sphq": sph,
                "spbq": spb,
                "b1q": b1m,
                "b2q": b2m,
            }
        )
    return in_maps


def _run(in_maps, cfg=DEFAULT_CFG, trace=False, tmpdir=None):
    key = str(sorted(cfg.items()))
    if key not in _CACHE:
        _CACHE[key] = _build(cfg)
    return run_bass_kernel_spmd(
        _CACHE[key],
        in_maps,
        core_ids=list(range(NCORE)),
        trace=trace,
        tmpdir=tmpdir,
    )


_LAST = {}


def kernel(x, W1, b1, beta, W2, b2):
    in_maps = _marshal(x, W1, b1, beta, W2, b2)
    trace = bool(os.environ.get("KERNEL_TRACE"))
    r = _run(in_maps, trace=trace, tmpdir=os.environ.get("KERNEL_TRACE_DIR"))
    _LAST["results"] = r
    outs = [
        r.results[c]["out"].astype(np.float32).reshape(GPC, C * L)
        for c in range(NCORE)
    ]
    return np.concatenate(outs, axis=0)


# revision 29
# speedup vs baseline: 1.1619x; 1.0929x over previous
"""Grouped per-sample MLP (conv1d groups=B) + GroupSwish + softmax, on 8 NeuronCores.

Data-parallel over the group axis B=256: 32 groups per core, processed in
8 quads of 4 groups. Per group g: h = W1[g] @ x[g] + b1[g]; GroupSwish;
o = W2[g] @ h + b2[g]; softmax over the flattened [C*L] logits.

Key design points (vs the fp32 baseline at ~312us):
  - x is shipped as fp8e4m3 (12.8 MB/core), W1/W2 as bf16. End-to-end
    rel_fro error ~6.5e-3 (numpy-simulated), well under the 2e-2 gate.
  - Contraction X=784 is split 7x112 (not 6x128+16) so every matmul is
    K<=128, M=32: uniform (128,32) PE tile mode -> no mode-switch drains.
  - 4 groups share the 128-wide PE array via column tiling: matmuls for
    the 4 groups of a quad write PSUM partition strips 32j..32j+31 and
    run concurrently (tile_position auto-derived from out.base_partition).
    Emission is chunk-outer / group-inner so the 4 strips stay busy.
  - Activations/DVE ops run on whole [128, 512] quads (DVE/ACT cost is
    per-free-dim-element, not per-partition, so 4 groups cost 1 group).
  - W2 per group is embedded as a [128, 32] column block (rows 32j..+31
    hold W2[g].T/1.1, rest zero) -> o quad in one (128,32)-mode pass;
    pad logit rows compute as exactly 0.
  - Softmax cross-partition sum + broadcast with two (128,32)-mode
    matmuls against constant selector matrices; garbage is never
    multiplied by 0 anywhere (no NaN paths).
  - x DMA per quad is split across both HWDGE rings (sync + scalar),
    one contiguous 7168B run per partition.
"""

import os
import numpy as np
import ml_dtypes
from contextlib import ExitStack

import concourse.mybir as mybir
import concourse.tile as tile
from concourse import bacc
from concourse.bass_utils import run_bass_kernel_spmd

B, X, Z, C, L = 256, 784, 32, 10, 512
NCORE = 8
GPC = B // NCORE  # 32 groups per core
NQ = GPC // 4  # 8 quads of 4 groups
NCH = 7  # contraction chunks
KP = X // NCH  # 112 rows per chunk
F32 = mybir.dt.float32
BF16 = mybir.dt.bfloat16
FP8 = mybir.dt.float8e4

NP_BF16 = ml_dtypes.bfloat16
NP_FP8 = ml_dtypes.float8_e4m3fn

DEFAULT_CFG = dict(
    x_bufs=8,
    s_bufs=3,
    h_bufs=3,
    o_bufs=2,
    warmup=20,
)

_CACHE: dict = {}


def _build(cfg=DEFAULT_CFG):
    nc = bacc.Bacc("TRN2", target_bir_lowering=False, debug=False)

    xm = nc.dram_tensor("xm", [NQ, 2, 128, 3 * 4 * L], FP8, kind="ExternalInput").ap()
    xtl = nc.dram_tensor("xtl", [NQ, 16, 4 * L], FP8, kind="ExternalInput").ap()
    w1m = nc.dram_tensor("w1m", [128, NQ * 4 * 6 * Z], BF16, kind="ExternalInput").ap()
    w1t = nc.dram_tensor("w1t", [KP, NQ * 4 * Z], BF16, kind="ExternalInput").ap()
    w2c = nc.dram_tensor("w2c", [128, NQ * 4 * Z], BF16, kind="ExternalInput").ap()
    onest = nc.dram_tensor("onest", [128, 4 * Z], BF16, kind="ExternalInput").ap()
    sphq = nc.dram_tensor("sphq", [128, NQ], F32, kind="ExternalInput").ap()
    spbq = nc.dram_tensor("spbq", [128, NQ], F32, kind="ExternalInput").ap()
    b1q = nc.dram_tensor("b1q", [128, NQ], F32, kind="ExternalInput").ap()
    b2q = nc.dram_tensor("b2q", [128, NQ], F32, kind="ExternalInput").ap()
    out = nc.dram_tensor("out", [GPC, C, L], BF16, kind="ExternalOutput").ap()

    with tile.TileContext(nc) as tc, ExitStack() as ctx:
        consts = ctx.enter_context(tc.tile_pool(name="consts", bufs=1))
        xpool = ctx.enter_context(tc.tile_pool(name="x", bufs=cfg["x_bufs"]))
        spool = ctx.enter_context(tc.tile_pool(name="act", bufs=cfg["s_bufs"]))
        hps = ctx.enter_context(tc.tile_pool(name="hps", bufs=cfg["h_bufs"], space="PSUM"))
        ops = ctx.enter_context(tc.tile_pool(name="ops", bufs=cfg["o_bufs"], space="PSUM"))
        tps = ctx.enter_context(tc.tile_pool(name="tps", bufs=2, space="PSUM"))

        w2t = consts.tile([128, NQ * 4 * Z], BF16, name="w2t")
        nc.gpsimd.dma_start(w2t[:], w2c)
        ot = consts.tile([128, 4 * Z], BF16, name="ot")
        nc.gpsimd.dma_start(ot[:], onest)
        spht = consts.tile([128, NQ], F32, name="spht")
        nc.gpsimd.dma_start(spht[:], sphq)
        spbt = consts.tile([128, NQ], F32, name="spbt")
        nc.gpsimd.dma_start(spbt[:], spbq)
        b1t = consts.tile([128, NQ], F32, name="b1t")
        nc.gpsimd.dma_start(b1t[:], b1q)
        b2t = consts.tile([128, NQ], F32, name="b2t")
        nc.gpsimd.dma_start(b2t[:], b2q)

        # all of W1 up front (1.6 MB): one DMA, first in the sync ring
        wall = consts.tile([128, NQ * 4 * 6 * Z], BF16, name="wall")
        nc.sync.dma_start(wall[:], w1m)
        wtail = consts.tile([128, NQ * 4 * Z], BF16, name="wtail")
        nc.scalar.dma_start(wtail[:KP, :], w1t)

        # static ping-pong buffers for the 16-row tail chunk: rows 16..111
        # stay zero so the tail matmul can run K=112 in (128,32) mode.
        tailbs = []
        for i in range(3):
            tb = consts.tile([128, 4 * L], FP8, name=f"tailb{i}")
            nc.vector.memset(tb[:], 0.0)
            tailbs.append(tb)

        # PE warm-up: ~20 dummy matmuls on uninitialized SBUF into a
        # scratch PSUM bank. No deps -> runs immediately; pushes the HAM
        # past its 4096-cycle activity window so real matmuls run at
        # 2.4 GHz instead of 1.2.
        scr = consts.tile([128, L], FP8, name="scr")
        nc.vector.memset(scr[:], 0.5)
        scw = consts.tile([128, Z], BF16, name="scw")
        nc.vector.memset(scw[:], 0.5)
        wps = ctx.enter_context(tc.tile_pool(name="wps", bufs=1, space="PSUM"))
        warm = wps.tile([Z, L], F32, name="warm")
        for i in range(cfg.get("warmup", 20)):
            nc.tensor.matmul(warm[:], scw[:], scr[:], start=True, stop=True)

        sws = {}  # q -> swish tile
        expos = {}  # q -> (expo, esb)

        def stage1(q):
            """loads + W1 matmuls + GroupSwish for quad q."""
            # main x: chunks 0-5 over all 128 partitions, half per HWDGE
            # ring, one contiguous 6144B run per partition per DMA.
            # col layout (c, j, l): rhs for (c, j) = xt[:, (c*4+j)*L : +L].
            xt = xpool.tile([128, 6 * 4 * L], FP8, tag="xt", name=f"xt{q}")
            HB = 3 * 4 * L
            nc.sync.dma_start(xt[:, :HB], xm[q, 0])
            nc.scalar.dma_start(xt[:, HB:], xm[q, 1])
            tb = tailbs[q % 3]
            nc.sync.dma_start(tb[:16, :], xtl[q])

            h4 = hps.tile([128, L], F32, tag="h", name=f"h{q}")
            for c in range(NCH):
                for j in range(4):
                    if c < 6:
                        lhsT = wall[:, ((q * 4 + j) * 6 + c) * Z : ((q * 4 + j) * 6 + c + 1) * Z]
                        rhs = xt[:, (c * 4 + j) * L : (c * 4 + j + 1) * L]
                    else:
                        lhsT = wtail[:KP, (q * 4 + j) * Z : (q * 4 + j + 1) * Z]
                        rhs = tb[:KP, j * L : (j + 1) * L]
                    nc.tensor.matmul(
                        h4[32 * j : 32 * j + 32, :],
                        lhsT,
                        rhs,
                        start=(c == 0),
                        stop=(c == NCH - 1),
                        tile_position=(0, 32 * j),
                    )

            t = spool.tile([128, L], BF16, tag="t", name=f"t{q}")
            nc.scalar.activation(
                t[:],
                h4[:],
                mybir.ActivationFunctionType.Tanh,
                bias=spbt[:, q : q + 1],
                scale=spht[:, q : q + 1],
            )
            u = spool.tile([128, L], BF16, tag="u", name=f"u{q}")
            nc.vector.tensor_scalar(
                u[:],
                h4[:],
                b1t[:, q : q + 1],
                0.5,
                op0=mybir.AluOpType.add,
                op1=mybir.AluOpType.mult,
            )
            sw = spool.tile([128, L], BF16, tag="sw", name=f"sw{q}")
            nc.vector.scalar_tensor_tensor(
                sw[:],
                t[:],
                1.0,
                u[:],
                op0=mybir.AluOpType.add,
                op1=mybir.AluOpType.mult,
            )
            sws[q] = sw

        def stage2(q):
            """W2 matmuls + exp for quad q (emitted one quad later)."""
            o4 = ops.tile([128, L], F32, tag="o", name=f"o{q}")
            sw = sws.pop(q)
            for j in range(4):
                g = 4 * q + j
                nc.tensor.matmul(
                    o4[32 * j : 32 * j + 32, :],
                    w2t[:, g * Z : (g + 1) * Z],
                    sw[:],
                    start=True,
                    stop=True,
                    tile_position=(0, 32 * j),
                )
            expo = spool.tile([128, L], BF16, tag="expo", name=f"e{q}")
            esum = spool.tile([128, 1], F32, tag="esum", name=f"es{q}")
            nc.scalar.activation(
                expo[:],
                o4[:],
                mybir.ActivationFunctionType.Exp,
                bias=b2t[:, q : q + 1],
                scale=1.0,
                accum_out=esum[:],
            )
            esb = spool.tile([128, 1], BF16, tag="esb", name=f"eb{q}")
            nc.vector.tensor_copy(esb[:], esum[:])
            expos[q] = (expo, esb)

        def stage3(q):
            """softmax normalization + store for quad q (two quads later)."""
            expo, esb = expos.pop(q)
            totb = tps.tile([128, 1], F32, tag="tb", name=f"tot{q}")
            for j in range(4):
                nc.tensor.matmul(
                    totb[32 * j : 32 * j + 32, :],
                    ot[:, 32 * j : 32 * j + 32],
                    esb[:],
                    start=True,
                    stop=True,
                    tile_position=(0, 32 * j),
                )
            invb = spool.tile([128, 1], F32, tag="invb", name=f"iv{q}")
            nc.vector.reciprocal(invb[:], totb[:])
            res = spool.tile([128, L], BF16, tag="res", name=f"r{q}")
            nc.vector.tensor_scalar_mul(res[:], expo[:], invb[:])
            for j in range(4):
                nc.gpsimd.dma_start(out[4 * q + j], res[32 * j : 32 * j + 10, :])

        for q in range(NQ):
            stage1(q)
            if q >= 1:
                stage2(q - 1)
            if q >= 2:
                stage3(q - 2)
        stage2(NQ - 1)
        stage3(NQ - 2)
        stage3(NQ - 1)

    nc.compile()
    return nc


def _marshal(x, W1, b1, beta, W2, b2):
    """Full inputs -> list of per-core input dicts (all layouts hardcoded)."""
    # x: [1, B*X, L] -> [B, X, L] fp8. Main chunks c<6 (rows 128c+p), tail
    # rows 768..783. xm[q, h, p, (c', j, l)] = x[4q+j, 128*(3h+c')+p, l]
    xg = np.asarray(x, dtype=np.float32).reshape(B, X, L)
    x8f = xg.astype(NP_FP8)
    xmain = x8f[:, : 6 * 128].reshape(B // 4, 4, 2, 3, 128, L).transpose(0, 2, 4, 3, 1, 5)
    xmain = np.ascontiguousarray(xmain).reshape(B // 4, 2, 128, 3 * 4 * L)
    xtail = x8f[:, 6 * 128 :].reshape(B // 4, 4, 16, L).transpose(0, 2, 1, 3)
    xtail = np.ascontiguousarray(xtail).reshape(B // 4, 16, 4 * L)

    # W1: [B, Z, X] -> main lhsT [128, (quad, j, c<6, z)] bf16; tail
    # lhsT [KP, (quad, j, z)] with rows 16..111 zero.
    w1T = np.asarray(W1, dtype=np.float32).transpose(0, 2, 1)  # [B, X, Z]
    w1main = w1T[:, : 6 * 128].reshape(B // 4, 4, 6, 128, Z).transpose(3, 0, 1, 2, 4)
    w1main = np.ascontiguousarray(w1main).astype(NP_BF16).reshape(128, (B // 4) * 4 * 6 * Z)
    w1tail = np.zeros((KP, B, Z), dtype=NP_BF16)
    w1tail[:16] = w1T[:, 6 * 128 :].transpose(1, 0, 2).astype(NP_BF16)
    w1tail = w1tail.reshape(KP, B * Z)

    # W2 blockdiag: w2c[32j+z, g*Z+c-block] = W2[g, c, z]/1.1 (per core below)
    w2s = (np.asarray(W2, dtype=np.float32) * np.float32(1.0 / 1.1)).transpose(0, 2, 1)  # [B, Z, C]

    onest = np.zeros((128, 4 * Z), dtype=NP_BF16)
    for j in range(4):
        onest[32 * j : 32 * j + C, 32 * j : 32 * j + 32] = NP_BF16(1.0)

    b1f = np.asarray(b1, dtype=np.float32)
    b2f = np.asarray(b2, dtype=np.float32)
    spf = np.log1p(np.exp(np.asarray(beta, dtype=np.float64))).astype(np.float32)

    in_maps = []
    for core in range(NCORE):
        g0 = core * GPC
        sq = slice(core * NQ, (core + 1) * NQ)

        w2core = np.zeros((128, NQ * 4 * Z), dtype=np.float32)
        sph = np.zeros((128, NQ), dtype=np.float32)
        spb = np.zeros((128, NQ), dtype=np.float32)
        b1m = np.zeros((128, NQ), dtype=np.float32)
        b2m = np.zeros((128, NQ), dtype=np.float32)
        for q in range(NQ):
            for j in range(4):
                g = g0 + 4 * q + j
                w2core[32 * j : 32 * j + Z, (4 * q + j) * Z : (4 * q + j) * Z + C] = w2s[g]
                sph[32 * j : 32 * j + Z, q] = 0.5 * spf[g]
                spb[32 * j : 32 * j + Z, q] = 0.5 * spf[g] * b1f[g]
                b1m[32 * j : 32 * j + Z, q] = b1f[g]
                b2m[32 * j : 32 * j + C, q] = b2f[g]

        wstep = NQ * 4 * 6 * Z
        tstep = NQ * 4 * Z
        in_maps.append(
            {
                "xm": xmain[sq],
                "xtl": xtail[sq],
                "w1m": np.ascontiguousarray(
                    w1main[:, core * wstep : (core + 1) * wstep]
                ),
                "w1t": np.ascontiguousarray(
                    w1tail[:, core * tstep : (core + 1) * tstep]
                ),
                "w2c": w2core.astype(NP_BF16),
                "onest": onest,
                "sphq": sph,
                "spbq": spb,
                "b1q": b1m,
                "b2q": b2m,
            }
        )
    return in_maps


def _run(in_maps, cfg=DEFAULT_CFG, trace=False, tmpdir=None):
    key = str(sorted(cfg.items()))
    if key not in _CACHE:
        _CACHE[key] = _build(cfg)
    return run_bass_kernel_spmd(
        _CACHE[key],
        in_maps,
        core_ids=list(range(NCORE)),
        trace=trace,
        tmpdir=tmpdir,
    )


_LAST = {}


def kernel(x, W1, b1, beta, W2, b2):
    in_maps = _marshal(x, W1, b1, beta, W2, b2)
    trace = bool(os.environ.get("KERNEL_TRACE"))
    r = _run(in_maps, trace=trace, tmpdir=os.environ.get("KERNEL_TRACE_DIR"))
    _LAST["results"] = r
    outs = [
        r.results[c]["out"].astype(np.float32).reshape(GPC, C * L)
        for c in range(NCORE)
    ]
    return np.concatenate(outs, axis=0)


# revision 36
# speedup vs baseline: 1.4151x; 1.2179x over previous
"""Grouped per-sample MLP (conv1d groups=B) + GroupSwish + softmax, on 8 NeuronCores.

Data-parallel over the group axis B=256: 32 groups per core, processed in
8 quads of 4 groups. Per group g: h = W1[g] @ x[g] + b1[g]; GroupSwish;
o = W2[g] @ h + b2[g]; softmax over the flattened [C*L] logits.

Key design points (vs the fp32 baseline at ~312us):
  - x is shipped as fp8e4m3 (12.8 MB/core), W1/W2 as bf16. End-to-end
    rel_fro error ~6.5e-3 (numpy-simulated), well under the 2e-2 gate.
  - Contraction X=784 is split 7x112 (not 6x128+16) so every matmul is
    K<=128, M=32: uniform (128,32) PE tile mode -> no mode-switch drains.
  - 4 groups share the 128-wide PE array via column tiling: matmuls for
    the 4 groups of a quad write PSUM partition strips 32j..32j+31 and
    run concurrently (tile_position auto-derived from out.base_partition).
    Emission is chunk-outer / group-inner so the 4 strips stay busy.
  - Activations/DVE ops run on whole [128, 512] quads (DVE/ACT cost is
    per-free-dim-element, not per-partition, so 4 groups cost 1 group).
  - W2 per group is embedded as a [128, 32] column block (rows 32j..+31
    hold W2[g].T/1.1, rest zero) -> o quad in one (128,32)-mode pass;
    pad logit rows compute as exactly 0.
  - Softmax cross-partition sum + broadcast with two (128,32)-mode
    matmuls against constant selector matrices; garbage is never
    multiplied by 0 anywhere (no NaN paths).
  - x DMA per quad is split across both HWDGE rings (sync + scalar),
    one contiguous 7168B run per partition.
"""

import os
import numpy as np
import ml_dtypes
from contextlib import ExitStack

import concourse.mybir as mybir
import concourse.tile as tile
from concourse import bacc
from concourse.bass_utils import run_bass_kernel_spmd

B, X, Z, C, L = 256, 784, 32, 10, 512
NCORE = 8
GPC = B // NCORE  # 32 groups per core
NQ = GPC // 4  # 8 quads of 4 groups
NCH = 7  # contraction chunks
KP = X // NCH  # 112 rows per chunk
F32 = mybir.dt.float32
BF16 = mybir.dt.bfloat16
FP8 = mybir.dt.float8e4

NP_BF16 = ml_dtypes.bfloat16
NP_FP8 = ml_dtypes.float8_e4m3fn

DEFAULT_CFG = dict(
    x_bufs=8,
    s_bufs=3,
    h_bufs=3,
    o_bufs=2,
    warmup=20,
)

_CACHE: dict = {}


def _build(cfg=DEFAULT_CFG):
    nc = bacc.Bacc("TRN2", target_bir_lowering=False, debug=False)

    xm = nc.dram_tensor("xm", [NQ, 2, 128, 3 * 4 * L], FP8, kind="ExternalInput").ap()
    xtl = nc.dram_tensor("xtl", [NQ, 16, 4 * L], FP8, kind="ExternalInput").ap()
    w1m = nc.dram_tensor("w1m", [128, NQ * 4 * 6 * Z], FP8, kind="ExternalInput").ap()
    w1t = nc.dram_tensor("w1t", [KP, NQ * 4 * Z], FP8, kind="ExternalInput").ap()
    w2c = nc.dram_tensor("w2c", [128, NQ * 4 * Z], BF16, kind="ExternalInput").ap()
    onest = nc.dram_tensor("onest", [128, 4 * Z], BF16, kind="ExternalInput").ap()
    sphq = nc.dram_tensor("sphq", [128, NQ], F32, kind="ExternalInput").ap()
    spbq = nc.dram_tensor("spbq", [128, NQ], F32, kind="ExternalInput").ap()
    b1q = nc.dram_tensor("b1q", [128, NQ], F32, kind="ExternalInput").ap()
    b2q = nc.dram_tensor("b2q", [128, NQ], F32, kind="ExternalInput").ap()
    out = nc.dram_tensor("out", [NQ, 128, L], BF16, kind="ExternalOutput").ap()

    with tile.TileContext(nc) as tc, ExitStack() as ctx:
        consts = ctx.enter_context(tc.tile_pool(name="consts", bufs=1))
        xpool = ctx.enter_context(tc.tile_pool(name="x", bufs=cfg["x_bufs"]))
        spool = ctx.enter_context(tc.tile_pool(name="act", bufs=cfg["s_bufs"]))
        hps = ctx.enter_context(tc.tile_pool(name="hps", bufs=cfg["h_bufs"], space="PSUM"))
        ops = ctx.enter_context(tc.tile_pool(name="ops", bufs=cfg["o_bufs"], space="PSUM"))
        tps = ctx.enter_context(tc.tile_pool(name="tps", bufs=2, space="PSUM"))

        w2t = consts.tile([128, NQ * 4 * Z], BF16, name="w2t")
        nc.gpsimd.dma_start(w2t[:], w2c)
        ot = consts.tile([128, 4 * Z], BF16, name="ot")
        nc.gpsimd.dma_start(ot[:], onest)
        spht = consts.tile([128, NQ], F32, name="spht")
        nc.gpsimd.dma_start(spht[:], sphq)
        spbt = consts.tile([128, NQ], F32, name="spbt")
        nc.gpsimd.dma_start(spbt[:], spbq)
        b1t = consts.tile([128, NQ], F32, name="b1t")
        nc.gpsimd.dma_start(b1t[:], b1q)
        b2t = consts.tile([128, NQ], F32, name="b2t")
        nc.gpsimd.dma_start(b2t[:], b2q)

        # all of W1 up front (0.8 MB fp8): one DMA, first in the sync ring
        wall = consts.tile([128, NQ * 4 * 6 * Z], FP8, name="wall")
        nc.sync.dma_start(wall[:], w1m)
        wtail = consts.tile([128, NQ * 4 * Z], FP8, name="wtail")
        nc.scalar.dma_start(wtail[:KP, :], w1t)

        # static ping-pong buffers for the 16-row tail chunk: rows 16..111
        # stay zero so the tail matmul can run K=112 in (128,32) mode.
        tailbs = []
        for i in range(3):
            tb = consts.tile([128, 4 * L], FP8, name=f"tailb{i}")
            nc.vector.memset(tb[:], 0.0)
            tailbs.append(tb)

        # PE warm-up: ~20 dummy matmuls on uninitialized SBUF into a
        # scratch PSUM bank. No deps -> runs immediately; pushes the HAM
        # past its 4096-cycle activity window so real matmuls run at
        # 2.4 GHz instead of 1.2.
        scr = consts.tile([128, L], FP8, name="scr")
        nc.vector.memset(scr[:], 0.5)
        scw = consts.tile([128, Z], BF16, name="scw")
        nc.vector.memset(scw[:], 0.5)
        wps = ctx.enter_context(tc.tile_pool(name="wps", bufs=1, space="PSUM"))
        warm = wps.tile([Z, L], F32, name="warm")
        for i in range(cfg.get("warmup", 20)):
            nc.tensor.matmul(warm[:], scw[:], scr[:], start=True, stop=True)

        sws = {}  # q -> swish tile
        expos = {}  # q -> (expo, esb)

        def stage1(q):
            """loads + W1 matmuls + GroupSwish for quad q."""
            # main x: chunks 0-5 over all 128 partitions, half per HWDGE
            # ring, one contiguous 6144B run per partition per DMA.
            # col layout (c, j, l): rhs for (c, j) = xt[:, (c*4+j)*L : +L].
            xt = xpool.tile([128, 6 * 4 * L], FP8, tag="xt", name=f"xt{q}")
            HB = 3 * 4 * L
            nc.sync.dma_start(xt[:, :HB], xm[q, 0])
            nc.scalar.dma_start(xt[:, HB:], xm[q, 1])
            tb = tailbs[q % 3]
            nc.gpsimd.dma_start(tb[:16, :], xtl[q])

            h4 = hps.tile([128, L], F32, tag="h", name=f"h{q}")
            for c in range(NCH):
                for j in range(4):
                    if c < 6:
                        lhsT = wall[:, ((q * 4 + j) * 6 + c) * Z : ((q * 4 + j) * 6 + c + 1) * Z]
                        rhs = xt[:, (c * 4 + j) * L : (c * 4 + j + 1) * L]
                    else:
                        lhsT = wtail[:KP, (q * 4 + j) * Z : (q * 4 + j + 1) * Z]
                        rhs = tb[:KP, j * L : (j + 1) * L]
                    nc.tensor.matmul(
                        h4[32 * j : 32 * j + 32, :],
                        lhsT,
                        rhs,
                        start=(c == 0),
                        stop=(c == NCH - 1),
                        tile_position=(0, 32 * j),
                    )

            t = spool.tile([128, L], BF16, tag="t", name=f"t{q}")
            nc.scalar.activation(
                t[:],
                h4[:],
                mybir.ActivationFunctionType.Tanh,
                bias=spbt[:, q : q + 1],
                scale=spht[:, q : q + 1],
            )
            u = spool.tile([128, L], BF16, tag="u", name=f"u{q}")
            nc.vector.tensor_scalar(
                u[:],
                h4[:],
                b1t[:, q : q + 1],
                0.5,
                op0=mybir.AluOpType.add,
                op1=mybir.AluOpType.mult,
            )
            sw = spool.tile([128, L], BF16, tag="sw", name=f"sw{q}")
            nc.vector.scalar_tensor_tensor(
                sw[:],
                t[:],
                1.0,
                u[:],
                op0=mybir.AluOpType.add,
                op1=mybir.AluOpType.mult,
            )
            sws[q] = sw

        def stage2(q):
            """W2 matmuls + exp for quad q (emitted one quad later)."""
            o4 = ops.tile([128, L], F32, tag="o", name=f"o{q}")
            sw = sws.pop(q)
            for j in range(4):
                g = 4 * q + j
                nc.tensor.matmul(
                    o4[32 * j : 32 * j + 32, :],
                    w2t[:, g * Z : (g + 1) * Z],
                    sw[:],
                    start=True,
                    stop=True,
                    tile_position=(0, 32 * j),
                )
            expo = spool.tile([128, L], BF16, tag="expo", name=f"e{q}")
            esum = spool.tile([128, 1], F32, tag="esum", name=f"es{q}")
            nc.scalar.activation(
                expo[:],
                o4[:],
                mybir.ActivationFunctionType.Exp,
                bias=b2t[:, q : q + 1],
                scale=1.0,
                accum_out=esum[:],
            )
            esb = spool.tile([128, 1], BF16, tag="esb", name=f"eb{q}")
            nc.vector.tensor_copy(esb[:], esum[:])
            expos[q] = (expo, esb)

        def stage3(q):
            """softmax normalization + store for quad q (two quads later)."""
            expo, esb = expos.pop(q)
            totb = tps.tile([128, 1], F32, tag="tb", name=f"tot{q}")
            for j in range(4):
                nc.tensor.matmul(
                    totb[32 * j : 32 * j + 32, :],
                    ot[:, 32 * j : 32 * j + 32],
                    esb[:],
                    start=True,
                    stop=True,
                    tile_position=(0, 32 * j),
                )
            invb = spool.tile([128, 1], F32, tag="invb", name=f"iv{q}")
            nc.vector.reciprocal(invb[:], totb[:])
            res = spool.tile([128, L], BF16, tag="res", name=f"r{q}")
            nc.vector.tensor_scalar_mul(res[:], expo[:], invb[:])
            nc.gpsimd.dma_start(out[q], res[:])

        for q in range(NQ):
            stage1(q)
            if q >= 1:
                stage2(q - 1)
            if q >= 2:
                stage3(q - 2)
        stage2(NQ - 1)
        stage3(NQ - 2)
        stage3(NQ - 1)

    nc.compile()
    return nc


def _marshal(x, W1, b1, beta, W2, b2):
    """Full inputs -> list of per-core input dicts (all layouts hardcoded)."""
    # x: [1, B*X, L] -> [B, X, L] fp8. Main chunks c<6 (rows 128c+p), tail
    # rows 768..783. xm[q, h, p, (c', j, l)] = x[4q+j, 128*(3h+c')+p, l]
    xg = np.asarray(x, dtype=np.float32).reshape(B, X, L)
    x8f = xg.astype(NP_FP8)
    xmain = x8f[:, : 6 * 128].reshape(B // 4, 4, 2, 3, 128, L).transpose(0, 2, 4, 3, 1, 5)
    xmain = np.ascontiguousarray(xmain).reshape(B // 4, 2, 128, 3 * 4 * L)
    xtail = x8f[:, 6 * 128 :].reshape(B // 4, 4, 16, L).transpose(0, 2, 1, 3)
    xtail = np.ascontiguousarray(xtail).reshape(B // 4, 16, 4 * L)

    # W1: [B, Z, X] -> main lhsT [128, (quad, j, c<6, z)] bf16; tail
    # lhsT [KP, (quad, j, z)] with rows 16..111 zero.
    w1T = np.asarray(W1, dtype=np.float32).transpose(0, 2, 1)  # [B, X, Z]
    w1main = w1T[:, : 6 * 128].reshape(B // 4, 4, 6, 128, Z).transpose(3, 0, 1, 2, 4)
    w1main = np.ascontiguousarray(w1main).astype(NP_FP8).reshape(128, (B // 4) * 4 * 6 * Z)
    w1tail = np.zeros((KP, B, Z), dtype=NP_FP8)
    w1tail[:16] = w1T[:, 6 * 128 :].transpose(1, 0, 2).astype(NP_FP8)
    w1tail = w1tail.reshape(KP, B * Z)

    # W2 blockdiag: w2c[32j+z, g*Z+c-block] = W2[g, c, z]/1.1 (per core below)
    w2s = (np.asarray(W2, dtype=np.float32) * np.float32(1.0 / 1.1)).transpose(0, 2, 1)  # [B, Z, C]

    onest = np.zeros((128, 4 * Z), dtype=NP_BF16)
    for j in range(4):
        onest[32 * j : 32 * j + C, 32 * j : 32 * j + 32] = NP_BF16(1.0)

    b1f = np.asarray(b1, dtype=np.float32)
    b2f = np.asarray(b2, dtype=np.float32)
    spf = np.log1p(np.exp(np.asarray(beta, dtype=np.float64))).astype(np.float32)

    in_maps = []
    for core in range(NCORE):
        g0 = core * GPC
        sq = slice(core * NQ, (core + 1) * NQ)

        w2core = np.zeros((128, NQ * 4 * Z), dtype=np.float32)
        sph = np.zeros((128, NQ), dtype=np.float32)
        spb = np.zeros((128, NQ), dtype=np.float32)
        b1m = np.zeros((128, NQ), dtype=np.float32)
        b2m = np.zeros((128, NQ), dtype=np.float32)
        for q in range(NQ):
            for j in range(4):
                g = g0 + 4 * q + j
                w2core[32 * j : 32 * j + Z, (4 * q + j) * Z : (4 * q + j) * Z + C] = w2s[g]
                sph[32 * j : 32 * j + Z, q] = 0.5 * spf[g]
                spb[32 * j : 32 * j + Z, q] = 0.5 * spf[g] * b1f[g]
                b1m[32 * j : 32 * j + Z, q] = b1f[g]
                b2m[32 * j : 32 * j + C, q] = b2f[g]

        wstep = NQ * 4 * 6 * Z
        tstep = NQ * 4 * Z
        in_maps.append(
            {
                "xm": xmain[sq],
                "xtl": xtail[sq],
                "w1m": np.ascontiguousarray(
                    w1main[:, core * wstep : (core + 1) * wstep]
                ),
                "w1t": np.ascontiguousarray(
                    w1tail[:, core * tstep : (core + 1) * tstep]
                ),
                "w2c": w2core.astype(NP_BF16),
                "onest": onest,
                "sphq": sph,
                "spbq": spb,
                "b1q": b1m,
                "b2q": b2m,
            }
        )
    return in_maps


def _run(in_maps, cfg=DEFAULT_CFG, trace=False, tmpdir=None):
    key = str(sorted(cfg.items()))
    if key not in _CACHE:
        _CACHE[key] = _build(cfg)
    return run_bass_kernel_spmd(
        _CACHE[key],
        in_maps,
        core_ids=list(range(NCORE)),
        trace=trace,
        tmpdir=tmpdir,
    )


_LAST = {}


def kernel(x, W1, b1, beta, W2, b2):
    in_maps = _marshal(x, W1, b1, beta, W2, b2)
    trace = bool(os.environ.get("KERNEL_TRACE"))
    r = _run(in_maps, trace=trace, tmpdir=os.environ.get("KERNEL_TRACE_DIR"))
    _LAST["results"] = r
    outs = [
        np.ascontiguousarray(
            r.results[c]["out"].reshape(NQ, 4, 32, L)[:, :, :C, :]
        )
        .astype(np.float32)
        .reshape(GPC, C * L)
        for c in range(NCORE)
    ]
    return np.concatenate(outs, axis=0)
